# revision 1
# baseline (speedup 1.0000x reference)
"""CubePad Trainium2 kernel (SBUF composition, manual semaphores).

Input  x: [12, 64, 256, 256] f32  (2 cubes x 6 faces, face order F,R,B,L,T,D)
Output y: [12, 64, 258, 258] f32  (1-px border gathered from neighboring faces)

Sharding: channel-parallel across 8 cores (8 channels each); every core holds
all 12 faces so cross-face border gathers stay local. Pure SPMD.

Strategy: compose each padded face in SBUF and write full 1032B rows. This
avoids both HW-measured failure modes of the DRAM->DRAM baseline: 4B border
descriptors (~44ns each, 55% of engine time) and half-rate DRAM->DRAM 1KB
descriptors (~120ns vs ~45ns one-directional).

4 pipelined stages (cube x 4-channel group), double-buffered SBUF. Raw bass
with a coarse semaphore schedule (per-stage phase barriers) because embedded
instruction waits are limited to ~2 slots by the core ISA:
  SP:   interior+strip loads (sem_L), straight-row strips DRAM->DRAM
  Pool: identity (sem_I), W-reversals + lateral border weaves (sem_P)
  DVE:  transpose-input staging (sem_F), PSUM->row/border weaves (sem_D)
  PE:   8 transposes per stage (sem_T)
  ACT:  all SBUF-sourced output writes (sem_wr)
"""

import numpy as np

N_CORES = 8
NF, C_FULL, H, W = 12, 64, 256, 256
C = C_FULL // N_CORES  # 8 channels per core
HP, WP = H + 2, W + 2
NC = 4  # channels per stage
STAGES = 4
LOADS = 19   # sem_L incs (x16) per stage
WRITES = 27  # sem_wr incs (x16) per stage (19 SBUF-sourced + 8 D2D)


def _build_bass():
    import concourse.bass as bass
    import concourse.mybir as mybir

    f32 = mybir.dt.float32
    nc = bass.Bass()
    x = nc.dram_tensor("x", [NF, C, H, W], f32, kind="ExternalInput")
    y = nc.dram_tensor("y", [NF, C, HP, WP], f32, kind="ExternalOutput")

    # ---- SBUF/PSUM allocation (2 parities for double buffering) ----
    def sb(name, shape):
        return nc.alloc_sbuf_tensor(name, shape, f32)

    ident = sb("ident", [128, 128])
    Y0 = [[sb(f"y0_{p}_{f}", [128, NC, WP]) for f in range(6)] for p in range(2)]
    Y1 = [[sb(f"y1_{p}_{f}", [128, NC, WP]) for f in range(6)] for p in range(2)]
    RVS = [sb(f"rvs_{p}", [4 * NC, W]) for p in range(2)]
    RVSo = [sb(f"rvso_{p}", [4 * NC, WP]) for p in range(2)]
    RCstr = [sb(f"rcstr_{p}", [2 * NC, W]) for p in range(2)]
    RCs = [sb(f"rcs_{p}", [2 * NC, W]) for p in range(2)]
    RCrev = [sb(f"rcrev_{p}", [2 * NC, W]) for p in range(2)]
    CRSa = [[sb(f"crsa{h}_{p}", [128, 2 * NC]) for h in range(2)] for p in range(2)]
    CRSb = [[sb(f"crsb{h}_{p}", [128, 2 * NC]) for h in range(2)] for p in range(2)]
    PSo = [sb(f"pso_{p}", [2 * NC, WP]) for p in range(2)]
    PSor = [sb(f"psor_{p}", [2 * NC, WP]) for p in range(2)]

    def ps(name, shape):
        return nc.alloc_psum_tensor(name, shape, f32)

    PSa = [ps(f"psa{h}", [2 * NC, 128]) for h in range(2)]
    PSb = [ps(f"psb{h}", [2 * NC, 128]) for h in range(2)]
    PRs = [ps(f"prs{h}", [128, 2 * NC]) for h in range(2)]
    PRr = [ps(f"prr{h}", [128, 2 * NC]) for h in range(2)]

    sem_L = nc.alloc_semaphore("sem_L")
    sem_I = nc.alloc_semaphore("sem_I")
    sem_F = nc.alloc_semaphore("sem_F")
    sem_T = nc.alloc_semaphore("sem_T")
    sem_D = nc.alloc_semaphore("sem_D")
    sem_P = nc.alloc_semaphore("sem_P")
    sem_wr = nc.alloc_semaphore("sem_wr")

    def stage_params(s):
        return (s % 2, 6 * (s // 2), slice(4 * (s % 2), 4 * (s % 2) + NC))

    with nc.Block() as block:

        @block.sync
        def _(sp):
            for s in range(STAGES):
                p, b, cs = stage_params(s)
                if s >= 2:
                    sp.wait_ge(sem_wr, (s - 1) * WRITES * 16)
                for f in range(6):
                    sp.dma_start(
                        Y0[p][f][:, :, 1:257],
                        x[b + f, cs, 0:128, :].transpose([1, 0, 2]),
                    ).then_inc(sem_L, 16)
                    sp.dma_start(
                        Y1[p][f][:, :, 1:257],
                        x[b + f, cs, 128:256, :].transpose([1, 0, 2]),
                    ).then_inc(sem_L, 16)
                # W-reversed row strips: [0:4]=(B,t)<-T r0, [4:8]=(B,d)<-D
                # r255, [8:12]=(T,t)<-B r0, [12:16]=(D,d)<-B r255
                sp.dma_start(RVS[p][0:4, :],
                             x[b + 4, cs, 0, :]).then_inc(sem_L, 16)
                sp.dma_start(RVS[p][4:8, :],
                             x[b + 5, cs, 255, :]).then_inc(sem_L, 16)
                sp.dma_start(
                    RVS[p][8:16, :],
                    x[b + 2, cs, 0:256:255, :].transpose([1, 0, 2]),
                ).then_inc(sem_L, 16)
                # row->col sources: straight L r0 / R r255; to-reverse L
                # r255 / R r0
                sp.dma_start(RCstr[p][0:4, :],
                             x[b + 3, cs, 0, :]).then_inc(sem_L, 16)
                sp.dma_start(RCstr[p][4:8, :],
                             x[b + 1, cs, 255, :]).then_inc(sem_L, 16)
                sp.dma_start(RCs[p][0:4, :],
                             x[b + 3, cs, 255, :]).then_inc(sem_L, 16)
                sp.dma_start(RCs[p][4:8, :],
                             x[b + 1, cs, 0, :]).then_inc(sem_L, 16)
                # straight row strips DRAM->DRAM (1KB descs + corner pixels):
                # (F,t)<-T r255, (D,t)<-F r255, (F,d)<-D r0, (T,d)<-F r0
                for (df, dr, sf, sr) in ((0, 0, 4, 255), (5, 0, 0, 255),
                                         (0, 257, 5, 0), (4, 257, 0, 0)):
                    sp.dma_start(y[b + df, cs, dr, 1:257],
                                 x[b + sf, cs, sr, :]).then_inc(sem_wr, 16)
                    with nc.allow_non_contiguous_dma(
                            reason="cubepad corner pixels"):
                        sp.dma_start(y[b + df, cs, dr, ::257],
                                     x[b + sf, cs, sr, ::255],
                                     ).then_inc(sem_wr, 16)

        @block.gpsimd
        def _(gp):
            gp.memset(ident[:, :], 0.0)
            gp.drain()
            gp.affine_select(
                out=ident[:, :], in_=ident[:, :],
                compare_op=mybir.AluOpType.not_equal, fill=1.0, base=0,
                pattern=[[-1, 128]], channel_multiplier=1,
            ).then_inc(sem_I, 1)
            for s in range(STAGES):
                p, b, cs = stage_params(s)
                gp.wait_ge(sem_L, (s + 1) * LOADS * 16)
                gp.tensor_copy(RVSo[p][:, 1:257], RVS[p][:, ::-1])
                gp.drain()
                gp.tensor_copy(RVSo[p][:, 0:1], RVSo[p][:, 1:2])
                gp.tensor_copy(RVSo[p][:, 257:258], RVSo[p][:, 256:257])
                # lateral borders: l: F<-L,R<-F,B<-R,L<-B (x col 255 = col
                # 256); r: F<-R,R<-B,B<-L,L<-F (x col 0 = col 1)
                last = None
                for Yh in (Y0[p], Y1[p]):
                    for i in range(4):
                        gp.tensor_copy(Yh[i][:, :, 0],
                                       Yh[(i + 3) % 4][:, :, 256])
                        gp.tensor_copy(Yh[i][:, :, 257],
                                       Yh[(i + 1) % 4][:, :, 1])
                gp.drain().then_inc(sem_P, 1)

        @block.vector
        def _(ve):
            for s in range(STAGES):
                p, b, cs = stage_params(s)
                ve.wait_ge(sem_L, (s + 1) * LOADS * 16)
                ve.tensor_copy(RCrev[p][:, :], RCs[p][:, ::-1])
                # CRS staging: CRSa [0:4]=T c0 (->L,t), [4:8]=D c255 (->R,d)
                #              CRSb [0:4]=T c255 (->R,t), [4:8]=D c0 (->L,d)
                for h, (Yh, CRSah, CRSbh) in enumerate(
                        ((Y0[p], CRSa[p][0], CRSb[p][0]),
                         (Y1[p], CRSa[p][1], CRSb[p][1]))):
                    ve.tensor_copy(CRSah[:, 0:4], Yh[4][:, :, 1])
                    ve.tensor_copy(CRSah[:, 4:8], Yh[5][:, :, 256])
                    ve.tensor_copy(CRSbh[:, 0:4], Yh[4][:, :, 256])
                    ve.tensor_copy(CRSbh[:, 4:8], Yh[5][:, :, 1])
                ve.drain().then_inc(sem_F, 1)
                ve.wait_ge(sem_T, s + 1)
                # composed t/d rows: PSo [0:4]=(L,t), [4:8]=(R,d) straight;
                # PSor [0:4]=(R,t), [4:8]=(L,d) reversed (half-swapped)
                ve.tensor_copy(PSo[p][:, 1:129], PSa[0][:, :])
                ve.tensor_copy(PSo[p][:, 129:257], PSa[1][:, :])
                ve.tensor_copy(PSor[p][:, 1:129], PSb[1][:, ::-1])
                ve.tensor_copy(PSor[p][:, 129:257], PSb[0][:, ::-1])
                ve.drain()
                for t in (PSo[p], PSor[p]):
                    ve.tensor_copy(t[:, 0:1], t[:, 1:2])
                    ve.tensor_copy(t[:, 257:258], t[:, 256:257])
                # T/D left/right borders from PR transposes
                for h, Yh in enumerate((Y0[p], Y1[p])):
                    ve.tensor_copy(Yh[4][:, :, 0], PRs[h][:, 0:4])
                    ve.tensor_copy(Yh[5][:, :, 257], PRs[h][:, 4:8])
                    ve.tensor_copy(Yh[5][:, :, 0], PRr[h][:, 0:4])
                    ve.tensor_copy(Yh[4][:, :, 257], PRr[h][:, 4:8])
                ve.drain().then_inc(sem_D, 1)

        @block.tensor
        def _(te):
            te.wait_ge(sem_I, 1)
            for s in range(STAGES):
                p, b, cs = stage_params(s)
                te.wait_ge(sem_F, s + 1)
                id8 = ident[0:8, 0:8]
                te.transpose(PSa[0][:, :], CRSa[p][0][:, :], ident[:, :])
                te.transpose(PSa[1][:, :], CRSa[p][1][:, :], ident[:, :])
                te.transpose(PSb[0][:, :], CRSb[p][0][:, :], ident[:, :])
                te.transpose(PSb[1][:, :], CRSb[p][1][:, :], ident[:, :])
                te.transpose(PRs[0][:, :], RCstr[p][:, 0:128], id8)
                te.transpose(PRs[1][:, :], RCstr[p][:, 128:256], id8)
                te.transpose(PRr[0][:, :], RCrev[p][:, 0:128], id8)
                te.transpose(PRr[1][:, :], RCrev[p][:, 128:256],
                             id8).then_inc(sem_T, 1)

        @block.scalar
        def _(ac):
            for s in range(STAGES):
                p, b, cs = stage_params(s)
                ac.wait_ge(sem_D, s + 1)
                ac.wait_ge(sem_P, s + 1)
                for f in range(6):
                    ac.dma_start(
                        y[b + f, cs, 1:129, :].transpose([1, 0, 2]),
                        Y0[p][f][:, :, :]).then_inc(sem_wr, 16)
                    ac.dma_start(
                        y[b + f, cs, 129:257, :].transpose([1, 0, 2]),
                        Y1[p][f][:, :, :]).then_inc(sem_wr, 16)
                # reversed row strips: B pair, (T,t), (D,d)
                ac.dma_start(
                    y[b + 2, cs, 0:258:257, :].transpose([1, 0, 2]),
                    RVSo[p][0:8, :]).then_inc(sem_wr, 16)
                ac.dma_start(y[b + 4, cs, 0, :],
                             RVSo[p][8:12, :]).then_inc(sem_wr, 16)
                ac.dma_start(y[b + 5, cs, 257, :],
                             RVSo[p][12:16, :]).then_inc(sem_wr, 16)
                # col-sourced strips: (L,t), (R,d), (R,t), (L,d)
                ac.dma_start(y[b + 3, cs, 0, :],
                             PSo[p][0:4, :]).then_inc(sem_wr, 16)
                ac.dma_start(y[b + 1, cs, 257, :],
                             PSo[p][4:8, :]).then_inc(sem_wr, 16)
                ac.dma_start(y[b + 1, cs, 0, :],
                             PSor[p][0:4, :]).then_inc(sem_wr, 16)
                ac.dma_start(y[b + 3, cs, 257, :],
                             PSor[p][4:8, :]).then_inc(sem_wr, 16)

    with nc.Block() as block2:

        @block2.sync
        def _(sp):
            sp.wait_ge(sem_wr, STAGES * WRITES * 16)

    nc.finalize()
    return nc


_NC_CACHE = None
_TRACE = False  # set by test.py to collect an NTFF profile
_LAST_EXEC_NS = None


def kernel(x: np.ndarray) -> np.ndarray:
    global _NC_CACHE, _LAST_EXEC_NS
    from concourse.bass_utils import run_bass_kernel_spmd

    assert x.shape == (NF, C_FULL, H, W) and x.dtype == np.float32
    if _NC_CACHE is None:
        _NC_CACHE = _build_bass()
    nc = _NC_CACHE

    in_maps = [
        {"x": np.ascontiguousarray(x[:, i * C:(i + 1) * C])} for i in range(N_CORES)
    ]
    res = run_bass_kernel_spmd(
        nc, in_maps, core_ids=list(range(N_CORES)), trace=_TRACE
    )
    _LAST_EXEC_NS = res.exec_time_ns
    out = np.empty((NF, C_FULL, HP, WP), dtype=np.float32)
    for i in range(N_CORES):
        out[:, i * C:(i + 1) * C] = res.results[i]["y"]
    return out

